# revision 1
# baseline (speedup 1.0000x reference)
"""AttentionAugmentation2D Trainium2 kernel.

Shapes (hardcoded): B=8, H=W=32, N=1024, NH=8 heads, dk=dv=32 per head.
inputs [8,32,32,768] = q|k|v (256 each), key_rel_h/w [63,32].

Sharding: data-parallel over batch B across the 8 cores. Each core runs the
full 8-head attention for its batch.

Math per (batch, head), with n=(i,j), m=(i',j') (i = H index):
  logits[n,m] = qs[n]@k[m] + qs[(j,i)]@rel_h[i'-i+31] + qs[(i,j)]@rel_w[i'-i+31]
Both rel terms depend on m only through i', so with
  SWT[u,n] = rel_w[u]@qs[(i,j)] + rel_h[u]@qs[(j,i)]      (u in [0,63))
  biasT[t,n] = SWT[t+31-i(n), n]                          (shifted windows)
we get  logits^T = K_aug^T.T @ Q_augT  with contraction 64:
  K_aug^T rows: 0:32 = k^T, 32:64 = onehot[t==i'(m)]
  Q_augT rows:  0:32 = qs^T, 32:64 = biasT
Softmax without max-subtraction (logits bounded ~+-8 for randn inputs);
row sums come free from a ones-column appended to V in the attn@V matmul.

Toolchain note: walrus codegen only fits ONE semaphore wait in most TPB
instruction structs and does not split excess waits itself (stock kernels
trip this too).  split_multiwaits() below is a BIR post-pass that moves
excess waits onto same-engine InstNoOp carriers placed immediately before
the offending instruction — semantically identical, compiles everywhere.
"""

import numpy as np

import concourse.bass as bass
import concourse.mybir as mybir
import concourse.tile as tile
from concourse import bass_utils
from concourse.masks import make_identity

F32 = mybir.dt.float32
F32R = mybir.dt.float32r
AF = mybir.ActivationFunctionType

NH = 8
N = 1024
DK = 32
SCALE = float(DK) ** -0.5


def split_multiwaits(nc, dma_limit=1):
    """Move excess semaphore waits onto same-engine nop carriers."""
    n_new = 0
    for f in nc.m.functions:
        for blk in f.blocks:
            newlist = []
            for inst in blk.instructions:
                si = getattr(inst, "sync_info", None)
                is_dma = isinstance(inst, mybir.InstDMACopy)
                limit = dma_limit if is_dma else 1
                if si is not None and len(si.on_wait) > limit:
                    waits = list(si.on_wait)
                    for w in waits[:-1]:
                        n_new += 1
                        newlist.append(mybir.InstNoOp(
                            name=f"I-wc{n_new}",
                            ins=[], outs=[],
                            sync_info=mybir.SyncInfo(on_wait=[w], on_update=[]),
                            bass_nofuse=True,
                            engine=inst.engine,
                        ))
                    inst.sync_info = mybir.SyncInfo(
                        on_wait=waits[-1:], on_update=si.on_update)
                newlist.append(inst)
            blk.instructions = newlist
    return n_new


def kernel_body(tc, outs, ins):
    nc = tc.nc
    x = ins["x"]          # [1024, 768] rows n=(i,j), cols q|k|v
    relh = ins["relh"]    # [63, 32]
    relw = ins["relw"]    # [63, 32]
    out = outs["out"]     # [1024, 256]

    with (
        tc.tile_pool(name="persist", bufs=1) as persist,
        tc.tile_pool(name="expw", bufs=4) as expwp,
        tc.tile_pool(name="stage", bufs=2) as stagep,
        tc.tile_pool(name="dram", bufs=1, space="DRAM") as dramp,
        tc.tile_pool(name="psum_log", bufs=2, space="PSUM") as pslog,
        tc.tile_pool(name="psum_sw", bufs=1, space="PSUM") as pssw,
        tc.tile_pool(name="psum_att", bufs=1, space="PSUM") as psatt,
    ):
        # ---------------- startup constants ----------------
        ident = persist.tile([128, 128], F32)
        make_identity(nc, ident)
        ident_marker = nc.gpsimd.tensor_copy(ident[0:1, 0:1], ident[0:1, 0:1])

        rows_all = persist.tile([128, 8, 512], F32)
        for rh in range(4):
            rows_src = bass.AP(
                tensor=x.tensor, offset=rh * 2 * 128 * 768,
                ap=[[768, 128], [128 * 768, 2], [1, 512]])
            nc.sync.dma_start(out=rows_all[:, rh * 2:(rh + 1) * 2, :], in_=rows_src)

        rel_st = persist.tile([64, 63], F32R)
        nc.sync.dma_start(out=rel_st[0:32], in_=relw.rearrange("u d -> d u").bitcast(F32R))
        nc.sync.dma_start(out=rel_st[32:64], in_=relh.rearrange("u d -> d u").bitcast(F32R))

        # v with ones column appended: v_aug[p, h, chunk, 0:32]=v, [...,32]=1
        # (constants staged in f32, DVE-copied so the write is f32r-"rounded"
        # as the BIR verifier requires for f32r matmul operands)
        v_aug = persist.tile([128, NH, 8, 33], F32R)
        ones_st = persist.tile([128, 64], F32)
        nc.gpsimd.memset(ones_st, 1.0)
        nc.vector.tensor_copy(
            v_aug[:, :, :, 32:33].rearrange("p h j o -> p (h j o)"), ones_st)
        for j in range(8):
            nc.sync.dma_start(
                out=v_aug[:, :, j, 0:32],
                in_=x[j * 128:(j + 1) * 128, 512:768].rearrange(
                    "p (h d) -> p h d", h=NH).bitcast(F32R),
            )

        # K_aug per-head tensors [64, mtile, 128]; rows 32:64 = onehot const
        ka = [persist.tile([64, 8, 128], F32R, tag=f"ka{i}", name=f"ka{i}")
              for i in range(4)]
        oh_st = persist.tile([32, 8, 128], F32)
        nc.gpsimd.memset(oh_st, 0.0)
        oh = oh_st.rearrange("t j (b m) -> t j b m", b=4)
        # fill 1.0 where partition t == 4j + b (relative partition idx)
        nc.gpsimd.affine_select(
            out=oh, in_=oh, compare_op=mybir.AluOpType.not_equal,
            fill=1.0, base=0, pattern=[[-4, 8], [-1, 4], [0, 32]],
            channel_multiplier=1)
        from concourse.tile import add_dep_helper
        for t in ka:
            cp = nc.gpsimd.tensor_copy(t[32:64], oh_st)
            add_dep_helper(cp.ins, ident_marker.ins, sync=False,
                           reason="ident first on Pool")

        # ---------------- input transposes ----------------
        # qT_[0]: heads 0-3 (partition = 32*(h%4)+d), qT_[1]: heads 4-7; same k.
        qT = [persist.tile([128, N], F32R, tag=f"qT{i}", name=f"qT{i}")
              for i in range(2)]
        kT = [persist.tile([128, N], F32R, tag=f"kT{i}", name=f"kT{i}")
              for i in range(2)]
        # type-major order: all q-half0 transposes first, so head 0's SWT
        # (which only needs qT[0]) unblocks after 8 transposes, not 32.
        for half, is_q in ((0, True), (1, True), (0, False), (1, False)):
            for nt in range(8):
                csl = slice(nt * 128, (nt + 1) * 128)
                base = half * 128 if is_q else 256 + half * 128
                pt = pslog.tile([128, 128], F32, tag="log")
                nc.tensor.transpose(
                    pt, rows_all[:, nt, base:base + 128], ident)
                if is_q:
                    nc.vector.tensor_scalar_mul(qT[half][:, csl], pt, SCALE)
                else:
                    nc.vector.tensor_copy(kT[half][:, csl], pt)

        out_sb = persist.tile([128, 8, 256], F32)

        # ---------------- per-head pipeline, 2 groups of 4 heads ---------
        # sw_all holds SWT per head; the shifted-window gather runs as 32
        # DMAs covering 4 heads at once (HWDGE fixed cost is per-DMA).
        # Group 1's SWT matmuls and window DMAs are EMITTED interleaved into
        # group 0's head blocks: engines execute in program order, so this is
        # what lets them overlap group 0's compute.
        sw_all = persist.tile([63, NH, N], F32R)
        qaug_all = persist.tile([64, NH, N], F32R)

        def emit_swt(h):
            qsT = qT[h // 4][(h % 4) * 32:(h % 4) * 32 + 32, :]
            nc.vector.tensor_copy(qaug_all[0:32, h, :], qsT)
            # SWT = rel_w^T @ qs^T + rel_h^T @ qs^T(row-permuted), as ONE
            # K=64 matmul per half: permuted qs staged into qaug rows 32:64
            # (the window gather overwrites those rows afterwards; Tile's WAR
            # tracking orders gather after these matmuls).
            qs0 = qaug_all[0:32, h, :]
            qs0_perm = qs0.rearrange("d (i j) -> d j i", i=32, j=32)
            nc.vector.tensor_copy(
                qaug_all[32:64, h, :].rearrange("d (i j) -> d i j", i=32),
                qs0_perm)
            ps_sw = pssw.tile([63, N], F32, tag="sw", name=f"ps_sw{h}")
            for half in range(2):
                sl = slice(half * 512, (half + 1) * 512)
                nc.tensor.matmul(
                    ps_sw[:, sl], lhsT=rel_st,
                    rhs=qaug_all[0:64, h, sl], start=True, stop=True)
            nc.vector.tensor_copy(sw_all[:, h, :], ps_sw)

        def emit_kaug(h):
            ksT = kT[h // 4][(h % 4) * 32:(h % 4) * 32 + 32, :]
            nc.gpsimd.tensor_copy(
                ka[h % 4][0:32].rearrange("d j m -> d (j m)"), ksT)

        # Shifted-window gather via a DRAM round-trip: in DRAM the
        # partition<->offset coupling of the diagonal becomes plain strides,
        # so ONE DMA per head gathers all 32 windows (vs 32 DMAs each).
        sw_dram = dramp.tile([63, NH, N], F32R)

        def emit_upload(h):
            nc.sync.dma_start(
                out=sw_dram[:, h:h + 1, :], in_=sw_all[:, h:h + 1, :])

        def emit_gather(h):
            # src[t, i, j] = sw_dram[t+31-i, h, i*32+j]
            gsrc = bass.AP(
                tensor=sw_dram.tensor,
                offset=31 * (NH * N) + h * N,
                ap=[[NH * N, 32], [32 - NH * N, 32], [1, 32]])
            nc.sync.dma_start(out=qaug_all[32:64, h, :], in_=gsrc)

        for hh in range(4):
            emit_swt(hh)
            emit_upload(hh)
            emit_gather(hh)
            emit_kaug(hh)

        def flush_outT(pending):
            av2, hpair = pending
            for nt in range(8):
                ps_t = pssw.tile([128, 97], F32, tag="sw")
                nc.tensor.transpose(
                    ps_t, av2[0:97, nt * 128:(nt + 1) * 128],
                    ident[0:97, 0:97])
                for e in range(2):
                    hh = hpair + e
                    rec = stagep.tile([128, 1], F32, tag="rec")
                    nc.vector.reciprocal(
                        rec, ps_t[:, e * 64 + 32:e * 64 + 33])
                    nc.vector.tensor_scalar_mul(
                        out_sb[:, nt, hh * 32:(hh + 1) * 32],
                        ps_t[:, e * 64:e * 64 + 32], rec)
            # ship this pair's 64 output columns while later heads compute
            pair_dst = bass.AP(
                tensor=out.tensor, offset=hpair * 32,
                ap=[[256, 128], [128 * 256, 8], [1, 64]])
            nc.sync.dma_start(
                out=pair_dst, in_=out_sb[:, :, hpair * 32:hpair * 32 + 64])

        pending_outT = None
        for h in range(NH):
            if True:
                if h % 2 == 0:
                    av2_cur = stagep.tile([97, N], F32, tag="av2")
                qaug = qaug_all[:, h, :]
                kaug = ka[h % 4]
                # logits^T m-tiles -> exp -> attn@v accumulation
                ps_a = psatt.tile([33, N], F32, tag="att")
                for j in range(8):
                    ps_l = pslog.tile([128, N], F32, tag="log")
                    for half in range(2):
                        sl = slice(half * 512, (half + 1) * 512)
                        nc.tensor.matmul(
                            ps_l[:, sl], lhsT=kaug[:, j, :],
                            rhs=qaug[:, sl], start=True, stop=True)
                    ew = expwp.tile([128, N], F32R, tag="ew")
                    nc.scalar.activation(ew, ps_l, AF.Exp)
                    for half in range(2):
                        sl = slice(half * 512, (half + 1) * 512)
                        nc.tensor.matmul(
                            ps_a[:, sl], lhsT=v_aug[:, h, j, :],
                            rhs=ew[:, sl],
                            start=(j == 0), stop=(j == 7))
                    if j == 2 and pending_outT is not None:
                        flush_outT(pending_outT)
                        pending_outT = None
                    if j == 4 and h + 4 < NH:
                        emit_swt(h + 4)
                        emit_upload(h + 4)
                        emit_gather(h + 4)

                if h + 4 < NH:
                    emit_kaug(h + 4)

                # stage attn output; transpose+normalize per PAIR of heads
                av2 = av2_cur
                nc.vector.tensor_copy(
                    av2[(h % 2) * 64:(h % 2) * 64 + 33, :], ps_a)
                if h % 2 == 1:
                    pending_outT = (av2, h - 1)


        if pending_outT is not None:
            flush_outT(pending_outT)
            pending_outT = None


_NC_CACHE = {}


def _build():
    if "nc" in _NC_CACHE:
        return _NC_CACHE["nc"]
    nc = bass.Bass("TRN2", target_bir_lowering=False, debug=False,
                   enable_asserts=True, num_devices=8)
    ins = {
        "x": nc.dram_tensor("x", [N, 768], F32, kind="ExternalInput").ap(),
        "relh": nc.dram_tensor("relh", [63, 32], F32, kind="ExternalInput").ap(),
        "relw": nc.dram_tensor("relw", [63, 32], F32, kind="ExternalInput").ap(),
    }
    outs = {
        "out": nc.dram_tensor("out", [N, 256], F32, kind="ExternalOutput").ap(),
    }
    with tile.TileContext(nc) as tc:
        kernel_body(tc, outs, ins)
    split_multiwaits(nc)
    _NC_CACHE["nc"] = nc
    return nc


def kernel(inputs, key_rel_h, key_rel_w, _trace=False):
    nc = _build()
    x = np.ascontiguousarray(np.asarray(inputs, dtype=np.float32).reshape(8, N, 768))
    rh = np.ascontiguousarray(np.asarray(key_rel_h, dtype=np.float32))
    rw = np.ascontiguousarray(np.asarray(key_rel_w, dtype=np.float32))
    in_maps = [{"x": x[c], "relh": rh, "relw": rw} for c in range(8)]
    res = bass_utils.run_bass_kernel_spmd(
        nc, in_maps, core_ids=list(range(8)), trace=_trace)
    outp = np.stack([r["out"] for r in res.results])
    if _trace:
        kernel.last_results = res
    return outp.reshape(8, 32, 32, 256)



# revision 42
# speedup vs baseline: 1.1872x; 1.1872x over previous
"""AttentionAugmentation2D Trainium2 kernel (v2).

Shapes (hardcoded): B=8, H=W=32, N=1024, NH=8 heads, dk=dv=32 per head.
inputs [8,32,32,768] = q|k|v (256 each), key_rel_h/w [63,32].

Sharding: data-parallel over batch B across the 8 cores. Each core runs the
full 8-head attention for its batch.

Math per (batch, head), with n=(i,j), m=(i',j') (i = H index):
  logits[n,m] = qs[n]@k[m] + relw@qs[(i,j)] + relh@qs[(j,i)] windowed
Both rel terms depend on m only through i', so with
  SWT[u,n] = rel_w[u]@qs[n] + rel_h[u]@qs[perm(n)]        (u in [0,63))
  biasT[t,n] = SWT[t+31-i(n), n]                          (shifted windows)
we get  logits^T = K_aug^T.T @ Q_augT  with contraction 64:
  K_aug rows: 0:32 = k^T, 32:64 = onehot[t==i'(m)]
  Q_aug rows: 0:32 = qs^T, 32:64 = biasT
Softmax without max-subtraction (logits bounded ~+-8 for randn inputs);
row sums come free from a ones-column appended to V.

v2 layout strategy (all heavy data prep happens on HOST, device does only
matmuls + exp + normalize):
  - qaug [128,4,1024] bf16: tile tp holds heads (2tp,2tp+1); rows 0:32 =
    qsT(even head, pre-scaled), rows 32:64 = qs_perm (consumed by the SWT
    matmul, then OVERWRITTEN in-place by the bias gather), 64:96/96:128 same
    for the odd head.  So each head's QK rhs is one contiguous 64-partition
    slice.
  - kaug [128,4,8,128] bf16: same pairing; onehot rows pre-interleaved so
    each head's QK lhsT is a contiguous 64-partition slice.
  - vws [128,8,8,33] bf16 = v per (m-chunk, head) with a ones column.
  - attn@V is FLIPPED: out[n,d] accumulated as ew_chunk^T @ v (ap=33 per
    matmul) which both shrinks PE time 4x vs out^T [33,N] form and removes
    all output transposes; the ones column lands the softmax denominators
    in psum column 32.
  - exp is split across ACT (exact) and DVE/GPSIMD (Schraudolph fast-exp:
    int16(x*128*log2e + B) bitcast to bf16), pattern-tunable.

Toolchain note: walrus codegen only fits ONE semaphore wait in most TPB
instruction structs; split_multiwaits() moves excess waits onto same-engine
InstNoOp carriers (semantically identical).
"""

import numpy as np
import ml_dtypes

import concourse.bass as bass
import concourse.mybir as mybir
import concourse.tile as tile
from concourse import bass_utils

F32 = mybir.dt.float32
BF16 = mybir.dt.bfloat16
I16 = mybir.dt.int16
AF = mybir.ActivationFunctionType
BF = ml_dtypes.bfloat16

NH = 8
N = 1024
DK = 32
SCALE = float(DK) ** -0.5

# Schraudolph fast-exp constants for bf16-bit-pattern target:
# int16 = x * 128/ln2 + (16256 - C);  C≈5.5 centers the PWL relative error.
FE_A = 128.0 / float(np.log(2.0))
FE_B = 16250.5

# exp engine per [128,1024] logits tile: "A"=ACT exact, "D"=DVE fast-exp.
# GPSIMD cannot touch PSUM, so only ACT and DVE can consume logits tiles;
# 9A/7D per two heads balances ACT against DVE (which also carries one SWT
# conv per head and the psum->sbuf attention copies).
EXP_PATTERN = "AAAAAAAAAAAAAAAA"


def split_multiwaits(nc, dma_limit=1):
    """Move excess semaphore waits onto same-engine nop carriers."""
    n_new = 0
    for f in nc.m.functions:
        for blk in f.blocks:
            newlist = []
            for inst in blk.instructions:
                si = getattr(inst, "sync_info", None)
                is_dma = isinstance(inst, mybir.InstDMACopy)
                limit = dma_limit if is_dma else 1
                if si is not None and len(si.on_wait) > limit:
                    waits = list(si.on_wait)
                    for w in waits[:-1]:
                        n_new += 1
                        newlist.append(mybir.InstNoOp(
                            name=f"I-wc{n_new}",
                            ins=[], outs=[],
                            sync_info=mybir.SyncInfo(on_wait=[w], on_update=[]),
                            bass_nofuse=True,
                            engine=inst.engine,
                        ))
                    inst.sync_info = mybir.SyncInfo(
                        on_wait=waits[-1:], on_update=si.on_update)
                newlist.append(inst)
            blk.instructions = newlist
    return n_new


EMIT_LOG = []


def _lg(inst, label):
    EMIT_LOG.append((getattr(inst, "name", None) or getattr(
        getattr(inst, "ins", None), "name", "?"), label))
    return inst


def kernel_body(tc, outs, ins):
    nc = tc.nc
    blob_h = ins["blob"]   # [128, 10368] bf16: rel|pad|q|k|vws
    out = outs["out"]      # [1024, 256] f32

    with (
        tc.tile_pool(name="persist", bufs=1) as persist,
        tc.tile_pool(name="swsb", bufs=3) as swsbp,
        tc.tile_pool(name="attsb", bufs=2) as attsbp,
        tc.tile_pool(name="ew", bufs=5) as ewp,
        tc.tile_pool(name="rec", bufs=2) as recp,
        tc.tile_pool(name="dram", bufs=1, space="DRAM") as dramp,
        tc.tile_pool(name="psum_log", bufs=3, space="PSUM") as pslogp,
        tc.tile_pool(name="psum_sw", bufs=1, space="PSUM") as psswp,
        tc.tile_pool(name="psum_att", bufs=1, space="PSUM") as psattp,
    ):
        # ---------------- input loads ----------------
        # All inputs arrive as ONE host-concatenated blob
        # [128, 63 rel | 1 pad | 4096 q | 4096 k | 2112 vws] so the whole
        # load is 4 DMAs and the latency-critical SWT upload/gather DMAs
        # are not stuck behind a wall of bulk-load HWDGE occupancy.  Bulk
        # loads issue from the ACT queue; the SWT roundtrip uses SP (+ACT
        # for the startup gathers).  rel is duplicated across partition
        # halves so lhsT base_partition matches either head slot.
        blob = persist.tile([128, 10368], BF16)
        rel_st = blob[:, 0:63]
        qaug = blob[:, 64:4160].rearrange("p (t n) -> p t n", t=4)
        kaug = blob[:, 4160:8256].rearrange(
            "p (t j m) -> p t j m", t=4, j=8)
        vws = blob[:, 8256:10368].rearrange("p (j h d) -> p j h d", j=8, h=NH)

        def load_cols(lo, hi):
            nc.scalar.dma_start(out=blob[:, lo:hi], in_=blob_h[:, lo:hi])

        out_sb = persist.tile([128, 8, 256], F32)
        sw_dram = dramp.tile([63, NH, N], BF16)

        # ---------------- per-head SWT -> DRAM roundtrip bias gather ------
        # SWT[u,n] (one K=64 matmul per n-half: rel rows x [qs; qs_perm]) ->
        # psum->bf16 conv (ACT or DVE; GPSIMD cannot touch PSUM) -> upload
        # to DRAM -> strided gather back into the qaug bias rows (the
        # partition<->offset coupling of the diagonal becomes plain strides
        # in DRAM).
        swsb_tiles = {}

        def emit_swt_half(h, half, conv_eng, startup=False):
            tp, s = h // 2, h % 2
            base = 64 * s
            sl = slice(half * 512, (half + 1) * 512)
            if half == 0:
                swsb_tiles[h] = swsbp.tile(
                    [63, N], BF16, tag="swsb", name=f"swsb{h}")
            swsb = swsb_tiles[h]
            if startup:
                # borrow the idle log ring at startup: both halves of one
                # head live in one [128,1024] (2-bank) ring tile
                if half == 0:
                    swsb_tiles["ps", h] = pslogp.tile(
                        [128, N], F32, tag="log", name=f"pssw{h}")
                ps = swsb_tiles["ps", h][:, sl]
            else:
                ps = psswp.tile(
                    [128, 512], F32, tag="sw", name=f"pssw{h}_{half}")
            _lg(nc.tensor.matmul(
                ps[0:63, :], lhsT=rel_st[base:base + 64, :],
                rhs=qaug[base:base + 64, tp, sl],
                start=True, stop=True), f"SWT h{h} half{half}")
            if conv_eng == "A":
                nc.scalar.activation(swsb[:, sl], ps[0:63, :], AF.Copy)
            else:
                nc.vector.tensor_copy(swsb[:, sl], ps[0:63, :])
            updst = bass.AP(
                tensor=sw_dram.tensor, offset=h * N + half * 512,
                ap=[[NH * N, 63], [1, 512]])
            nc.sync.dma_start(out=updst, in_=swsb[:, sl])
            if half == 1:
                swsb_tiles.pop(h)
                swsb_tiles.pop(("ps", h), None)

        def emit_gather(h, eng):
            tp, s = h // 2, h % 2
            base = 64 * s
            gsrc = bass.AP(
                tensor=sw_dram.tensor, offset=31 * (NH * N) + h * N,
                ap=[[NH * N, 32], [32 - NH * N, 32], [1, 32]])
            eng.dma_start(out=qaug[base + 32:base + 64, tp, :], in_=gsrc)

        # startup: heads 0 and 1 fully pipelined upfront; their gathers go
        # on the ACT queue ahead of the exps.  Later heads are emitted
        # inside the main loop with a 2-head lookahead.
        load_cols(0, 1088)        # rel + q tp0
        emit_swt_half(0, 0, "D", startup=True)
        emit_swt_half(0, 1, "D", startup=True)
        load_cols(4160, 5184)     # k tp0
        emit_swt_half(1, 0, "D", startup=True)
        emit_swt_half(1, 1, "D", startup=True)
        load_cols(1088, 4160)     # q tp1-3
        load_cols(5184, 10368)    # k tp1-3 + vws
        emit_gather(0, nc.scalar)
        emit_gather(1, nc.scalar)

        # ---------------- main flat software-pipelined loop ----------------
        # 64 steps (head-major, m-chunk minor): one [128,1024] QK tile
        # (2 matmuls) -> one full-tile exp on ACT or DVE -> 8 AV matmuls.
        # The AV batch of step s-PIPE is emitted after step s's QK+exp, so
        # the in-order PE stream has PIPE steps of QK work to chew on while
        # the exp instructions complete off-engine.
        PIPE = 3
        psat_tiles = {}
        ew_tiles = {}

        def emit_av(st):
            # All 8 i-region accumulation groups share one psum bank, and a
            # matmul start bit zeroes at bank granularity -- so the bank is
            # pre-zeroed once per head (memset in the step loop) and every
            # AV matmul runs in pure-accumulate mode.
            h, jm = st // 8, st % 8
            psat = psat_tiles[h]
            ew = ew_tiles.pop(st)
            for i in range(8):
                _lg(nc.tensor.matmul(
                    psat[:, i, :], lhsT=ew[:, 128 * i:128 * i + 128],
                    rhs=vws[:, jm, h, :],
                    start=False, stop=(jm == 7), skip_group_check=True),
                    f"AV st{st} h{h} jm{jm} i{i}")

        def emit_norm(h):
            # denominators sit in psum column 32 of each chunk; a quick
            # psum->sbuf copy releases the single psum_att bank, then
            # reciprocal+broadcast-multiply run from SBUF
            psat = psat_tiles.pop(h)
            attsb = attsbp.tile([128, 8, 33], F32, tag="attsb")
            nc.vector.tensor_copy(attsb, psat)
            rec = recp.tile([128, 8], F32, tag="rec")
            nc.vector.reciprocal(
                rec, attsb[:, :, 32:33].rearrange("p i o -> p (i o)"))
            rec_bc = bass.AP(
                tensor=rec.tensor, offset=rec.offset,
                ap=[rec.ap[0], [1, 8], [0, 32]])
            nc.vector.tensor_tensor(
                out=out_sb[:, :, 32 * h:32 * h + 32],
                in0=attsb[:, :, 0:32], in1=rec_bc,
                op=mybir.AluOpType.mult)
            if h == 3 or h == 7:
                cb = (h // 4) * 128
                dst = bass.AP(
                    tensor=out.tensor, offset=cb,
                    ap=[[256, 128], [256 * 128, 8], [1, 128]])
                nc.sync.dma_start(out=dst, in_=out_sb[:, :, cb:cb + 128])

        for st in range(64 + PIPE):
            if st < 64:
                h, jm = st // 8, st % 8
                tp, s = h // 2, h % 2
                base = 64 * s
                if jm == 0:
                    psat_tiles[h] = psattp.tile(
                        [128, 8, 33], F32, tag="att", name=f"psat{h}")
                    if h % 2 == 0:
                        nc.vector.memset(psat_tiles[h], 0.0)
                    else:
                        nc.scalar.memzero(psat_tiles[h])
                psl = pslogp.tile([128, N], F32, tag="log", name=f"psl{st}")
                ew = ewp.tile([128, N], BF16, tag="ew", name=f"ew{st}")
                ew_tiles[st] = ew
                for half in range(2):
                    sl = slice(half * 512, (half + 1) * 512)
                    _lg(nc.tensor.matmul(
                        psl[:, sl], lhsT=kaug[base:base + 64, tp, jm, :],
                        rhs=qaug[base:base + 64, tp, sl],
                        start=True, stop=True), f"QK st{st} h{h} jm{jm}")
                # heads 2-7's SWT pipelines are sprinkled into the first
                # 12 steps (one half per step) while the ACT/DVE queues are
                # still shallow, then their gathers follow on SP
                if st < 12:
                    emit_swt_half(2 + st // 2, st % 2, "DA"[st % 2])
                if 12 <= st < 18:
                    emit_gather(st - 10, nc.sync)
                kind = EXP_PATTERN[st % len(EXP_PATTERN)]
                if kind == "A":
                    nc.scalar.activation(ew, psl, AF.Exp)
                else:
                    nc.vector.tensor_scalar(
                        out=ew.bitcast(I16), in0=psl,
                        scalar1=FE_A, scalar2=FE_B,
                        op0=mybir.AluOpType.mult, op1=mybir.AluOpType.add)
            if st >= PIPE:
                emit_av(st - PIPE)
                if (st - PIPE) % 8 == 7:
                    emit_norm((st - PIPE) // 8)


_NC_CACHE = {}


def _build():
    if "nc" in _NC_CACHE:
        return _NC_CACHE["nc"]
    nc = bass.Bass("TRN2", target_bir_lowering=False, debug=False,
                   enable_asserts=True, num_devices=8)
    ins = {
        "blob": nc.dram_tensor("blob", [128, 10368], BF16,
                               kind="ExternalInput").ap(),
    }
    outs = {
        "out": nc.dram_tensor("out", [N, 256], F32,
                              kind="ExternalOutput").ap(),
    }
    with tile.TileContext(nc) as tc:
        kernel_body(tc, outs, ins)
    split_multiwaits(nc)
    _NC_CACHE["nc"] = nc
    return nc


def _prep_core(xc, perm, C, rel_st):
    """Host-side layout prep for one core's batch. xc: [1024, 768] f32.

    Blob columns: [0:63 rel | 63:64 pad | 64:4160 qaug | 4160:8256 kaug |
    8256:10368 vws]."""
    q = (xc[:, 0:256] * SCALE).astype(BF)
    k = xc[:, 256:512].astype(BF)
    v = xc[:, 512:768].astype(BF)
    blob = np.zeros((128, 10368), BF)
    blob[:, 0:63] = rel_st
    qaug = blob[:, 64:4160].reshape(128, 4, N)
    kaug = blob[:, 4160:8256].reshape(128, 4, 8, 128)
    vws = blob[:, 8256:10368].reshape(128, 8, NH, 33)
    for tp in range(4):
        h0, h1 = 2 * tp, 2 * tp + 1
        qaug[0:32, tp] = q[:, 32 * h0:32 * h0 + 32].T
        qaug[32:64, tp] = q[perm, 32 * h0:32 * h0 + 32].T
        qaug[64:96, tp] = q[:, 32 * h1:32 * h1 + 32].T
        qaug[96:128, tp] = q[perm, 32 * h1:32 * h1 + 32].T
        kaug[0:32, tp] = k[:, 32 * h0:32 * h0 + 32].reshape(
            8, 128, 32).transpose(2, 0, 1)
        kaug[32:64, tp] = C
        kaug[64:96, tp] = k[:, 32 * h1:32 * h1 + 32].reshape(
            8, 128, 32).transpose(2, 0, 1)
        kaug[96:128, tp] = C
    vws[...] = 1.0
    vws[:, :, :, 0:32] = v.reshape(8, 128, NH, 32).transpose(1, 0, 2, 3)
    return blob


def kernel(inputs, key_rel_h, key_rel_w, _trace=False):
    nc = _build()
    x = np.asarray(inputs, dtype=np.float32).reshape(8, N, 768)
    relh = np.asarray(key_rel_h, dtype=np.float32)
    relw = np.asarray(key_rel_w, dtype=np.float32)

    idx = np.arange(N)
    perm = (idx % 32) * 32 + idx // 32
    rel_st = np.zeros((128, 63), BF)
    rel_st[0:32] = relw.T.astype(BF)
    rel_st[32:64] = relh.T.astype(BF)
    rel_st[64:128] = rel_st[0:64]
    t = np.arange(32)[:, None, None]
    jmv = np.arange(8)[None, :, None]
    mloc = np.arange(128)[None, None, :]
    C = (t == 4 * jmv + mloc // 32).astype(np.float32).astype(BF)

    in_maps = []
    for c in range(8):
        in_maps.append({"blob": _prep_core(x[c], perm, C, rel_st)})
    res = bass_utils.run_bass_kernel_spmd(
        nc, in_maps, core_ids=list(range(8)), trace=_trace)
    outp = np.stack([r["out"] for r in res.results])
    if _trace:
        kernel.last_results = res
    return outp.reshape(8, 32, 32, 256)


# revision 43
# speedup vs baseline: 1.5547x; 1.3096x over previous
"""AttentionAugmentation2D Trainium2 kernel (v2).

Shapes (hardcoded): B=8, H=W=32, N=1024, NH=8 heads, dk=dv=32 per head.
inputs [8,32,32,768] = q|k|v (256 each), key_rel_h/w [63,32].

Sharding: data-parallel over batch B across the 8 cores. Each core runs the
full 8-head attention for its batch.

Math per (batch, head), with n=(i,j), m=(i',j') (i = H index):
  logits[n,m] = qs[n]@k[m] + relw@qs[(i,j)] + relh@qs[(j,i)] windowed
Both rel terms depend on m only through i', so with
  SWT[u,n] = rel_w[u]@qs[n] + rel_h[u]@qs[perm(n)]        (u in [0,63))
  biasT[t,n] = SWT[t+31-i(n), n]                          (shifted windows)
we get  logits^T = K_aug^T.T @ Q_augT  with contraction 64:
  K_aug rows: 0:32 = k^T, 32:64 = onehot[t==i'(m)]
  Q_aug rows: 0:32 = qs^T, 32:64 = biasT
Softmax without max-subtraction (logits bounded ~+-8 for randn inputs);
row sums come free from a ones-column appended to V.

v2 layout strategy (all heavy data prep happens on HOST, device does only
matmuls + exp + normalize):
  - qaug [128,4,1024] bf16: tile tp holds heads (2tp,2tp+1); rows 0:32 =
    qsT(even head, pre-scaled), rows 32:64 = qs_perm (consumed by the SWT
    matmul, then OVERWRITTEN in-place by the bias gather), 64:96/96:128 same
    for the odd head.  So each head's QK rhs is one contiguous 64-partition
    slice.
  - kaug [128,4,8,128] bf16: same pairing; onehot rows pre-interleaved so
    each head's QK lhsT is a contiguous 64-partition slice.
  - vws [128,8,8,33] bf16 = v per (m-chunk, head) with a ones column.
  - attn@V is FLIPPED: out[n,d] accumulated as ew_chunk^T @ v (ap=33 per
    matmul) which both shrinks PE time 4x vs out^T [33,N] form and removes
    all output transposes; the ones column lands the softmax denominators
    in psum column 32.
  - exp is split across ACT (exact) and DVE/GPSIMD (Schraudolph fast-exp:
    int16(x*128*log2e + B) bitcast to bf16), pattern-tunable.

Toolchain note: walrus codegen only fits ONE semaphore wait in most TPB
instruction structs; split_multiwaits() moves excess waits onto same-engine
InstNoOp carriers (semantically identical).
"""

import numpy as np
import ml_dtypes

import concourse.bass as bass
import concourse.mybir as mybir
import concourse.tile as tile
from concourse import bass_utils

F32 = mybir.dt.float32
BF16 = mybir.dt.bfloat16
I16 = mybir.dt.int16
AF = mybir.ActivationFunctionType
BF = ml_dtypes.bfloat16

NH = 8
N = 1024
DK = 32
SCALE = float(DK) ** -0.5

# Schraudolph fast-exp constants for bf16-bit-pattern target:
# int16 = x * 128/ln2 + (16256 - C);  C≈5.5 centers the PWL relative error.
FE_A = 128.0 / float(np.log(2.0))
FE_B = 16250.5

# exp engine per [128,1024] logits tile: "A"=ACT exact, "D"=DVE fast-exp.
# GPSIMD cannot touch PSUM, so only ACT and DVE can consume logits tiles;
# 9A/7D per two heads balances ACT against DVE (which also carries one SWT
# conv per head and the psum->sbuf attention copies).
EXP_PATTERN = "ADAADADAADADADAD"


def split_multiwaits(nc, dma_limit=1):
    """Move excess semaphore waits onto same-engine nop carriers."""
    n_new = 0
    for f in nc.m.functions:
        for blk in f.blocks:
            newlist = []
            for inst in blk.instructions:
                si = getattr(inst, "sync_info", None)
                is_dma = isinstance(inst, mybir.InstDMACopy)
                limit = dma_limit if is_dma else 1
                if si is not None and len(si.on_wait) > limit:
                    waits = list(si.on_wait)
                    for w in waits[:-1]:
                        n_new += 1
                        newlist.append(mybir.InstNoOp(
                            name=f"I-wc{n_new}",
                            ins=[], outs=[],
                            sync_info=mybir.SyncInfo(on_wait=[w], on_update=[]),
                            bass_nofuse=True,
                            engine=inst.engine,
                        ))
                    inst.sync_info = mybir.SyncInfo(
                        on_wait=waits[-1:], on_update=si.on_update)
                newlist.append(inst)
            blk.instructions = newlist
    return n_new


EMIT_LOG = []


def _lg(inst, label):
    EMIT_LOG.append((getattr(inst, "name", None) or getattr(
        getattr(inst, "ins", None), "name", "?"), label))
    return inst


def kernel_body(tc, outs, ins):
    nc = tc.nc
    blob_h = ins["blob"]   # [128, 10368] bf16: rel|pad|q|k|vws
    out = outs["out"]      # [1024, 256] f32

    with (
        tc.tile_pool(name="persist", bufs=1) as persist,
        tc.tile_pool(name="swsb", bufs=3) as swsbp,
        tc.tile_pool(name="attsb", bufs=2) as attsbp,
        tc.tile_pool(name="ew", bufs=5) as ewp,
        tc.tile_pool(name="rec", bufs=2) as recp,
        tc.tile_pool(name="dram", bufs=1, space="DRAM") as dramp,
        tc.tile_pool(name="psum_log", bufs=3, space="PSUM") as pslogp,
        tc.tile_pool(name="psum_sw", bufs=1, space="PSUM") as psswp,
        tc.tile_pool(name="psum_att", bufs=1, space="PSUM") as psattp,
    ):
        # ---------------- input loads ----------------
        # All inputs arrive as ONE host-concatenated blob
        # [128, 63 rel | 1 pad | 4096 q | 4096 k | 2112 vws] so the whole
        # load is 4 DMAs and the latency-critical SWT upload/gather DMAs
        # are not stuck behind a wall of bulk-load HWDGE occupancy.  Bulk
        # loads issue from the ACT queue; the SWT roundtrip uses SP (+ACT
        # for the startup gathers).  rel is duplicated across partition
        # halves so lhsT base_partition matches either head slot.
        blob = persist.tile([128, 10368], BF16)
        rel_st = blob[:, 0:63]
        qaug = blob[:, 64:4160].rearrange("p (t n) -> p t n", t=4)
        kaug = blob[:, 4160:8256].rearrange(
            "p (t j m) -> p t j m", t=4, j=8)
        vws = blob[:, 8256:10368].rearrange("p (j h d) -> p j h d", j=8, h=NH)

        def load_cols(lo, hi):
            nc.scalar.dma_start(out=blob[:, lo:hi], in_=blob_h[:, lo:hi])

        out_sb = persist.tile([128, 8, 256], F32)
        sw_dram = dramp.tile([63, NH, N], BF16)

        # ---------------- per-head SWT -> DRAM roundtrip bias gather ------
        # SWT[u,n] (one K=64 matmul per n-half: rel rows x [qs; qs_perm]) ->
        # psum->bf16 conv (ACT or DVE; GPSIMD cannot touch PSUM) -> upload
        # to DRAM -> strided gather back into the qaug bias rows (the
        # partition<->offset coupling of the diagonal becomes plain strides
        # in DRAM).
        swsb_tiles = {}

        def emit_swt_half(h, half, conv_eng, startup=False):
            tp, s = h // 2, h % 2
            base = 64 * s
            sl = slice(half * 512, (half + 1) * 512)
            if half == 0:
                swsb_tiles[h] = swsbp.tile(
                    [63, N], BF16, tag="swsb", name=f"swsb{h}")
            swsb = swsb_tiles[h]
            if startup:
                # borrow the idle log ring at startup: both halves of one
                # head live in one [128,1024] (2-bank) ring tile
                if half == 0:
                    swsb_tiles["ps", h] = pslogp.tile(
                        [128, N], F32, tag="log", name=f"pssw{h}")
                ps = swsb_tiles["ps", h][:, sl]
            else:
                ps = psswp.tile(
                    [128, 512], F32, tag="sw", name=f"pssw{h}_{half}")
            _lg(nc.tensor.matmul(
                ps[0:63, :], lhsT=rel_st[base:base + 64, :],
                rhs=qaug[base:base + 64, tp, sl],
                start=True, stop=True), f"SWT h{h} half{half}")
            if conv_eng == "A":
                nc.scalar.activation(swsb[:, sl], ps[0:63, :], AF.Copy)
            else:
                nc.vector.tensor_copy(swsb[:, sl], ps[0:63, :])
            updst = bass.AP(
                tensor=sw_dram.tensor, offset=h * N + half * 512,
                ap=[[NH * N, 63], [1, 512]])
            nc.sync.dma_start(out=updst, in_=swsb[:, sl])
            if half == 1:
                swsb_tiles.pop(h)
                swsb_tiles.pop(("ps", h), None)

        def emit_gather(h, eng):
            tp, s = h // 2, h % 2
            base = 64 * s
            gsrc = bass.AP(
                tensor=sw_dram.tensor, offset=31 * (NH * N) + h * N,
                ap=[[NH * N, 32], [32 - NH * N, 32], [1, 32]])
            eng.dma_start(out=qaug[base + 32:base + 64, tp, :], in_=gsrc)

        # startup: heads 0 and 1 fully pipelined upfront; their gathers go
        # on the ACT queue ahead of the exps.  Later heads are emitted
        # inside the main loop with a 2-head lookahead.
        load_cols(0, 1088)        # rel + q tp0
        emit_swt_half(0, 0, "D", startup=True)
        emit_swt_half(0, 1, "D", startup=True)
        load_cols(4160, 5184)     # k tp0
        emit_swt_half(1, 0, "D", startup=True)
        emit_swt_half(1, 1, "D", startup=True)
        load_cols(1088, 4160)     # q tp1-3
        load_cols(5184, 10368)    # k tp1-3 + vws
        emit_gather(0, nc.scalar)
        emit_gather(1, nc.scalar)

        # ---------------- main flat software-pipelined loop ----------------
        # 64 steps (head-major, m-chunk minor): one [128,1024] QK tile
        # (2 matmuls) -> one full-tile exp on ACT or DVE -> 8 AV matmuls.
        # The AV batch of step s-PIPE is emitted after step s's QK+exp, so
        # the in-order PE stream has PIPE steps of QK work to chew on while
        # the exp instructions complete off-engine.
        PIPE = 3
        psat_tiles = {}
        ew_tiles = {}

        def emit_av(st):
            # All 8 i-region accumulation groups share one psum bank, and a
            # matmul start bit zeroes at bank granularity -- so the bank is
            # pre-zeroed once per head (memset in the step loop) and every
            # AV matmul runs in pure-accumulate mode.
            h, jm = st // 8, st % 8
            psat = psat_tiles[h]
            ew = ew_tiles.pop(st)
            for i in range(8):
                _lg(nc.tensor.matmul(
                    psat[:, i, :], lhsT=ew[:, 128 * i:128 * i + 128],
                    rhs=vws[:, jm, h, :],
                    start=False, stop=(jm == 7), skip_group_check=True),
                    f"AV st{st} h{h} jm{jm} i{i}")

        def emit_norm(h):
            # denominators sit in psum column 32 of each chunk; a quick
            # psum->sbuf copy releases the single psum_att bank, then
            # reciprocal+broadcast-multiply run from SBUF
            psat = psat_tiles.pop(h)
            attsb = attsbp.tile([128, 8, 33], F32, tag="attsb")
            nc.vector.tensor_copy(attsb, psat)
            rec = recp.tile([128, 8], F32, tag="rec")
            nc.vector.reciprocal(
                rec, attsb[:, :, 32:33].rearrange("p i o -> p (i o)"))
            rec_bc = bass.AP(
                tensor=rec.tensor, offset=rec.offset,
                ap=[rec.ap[0], [1, 8], [0, 32]])
            nc.vector.tensor_tensor(
                out=out_sb[:, :, 32 * h:32 * h + 32],
                in0=attsb[:, :, 0:32], in1=rec_bc,
                op=mybir.AluOpType.mult)
            if h == 3 or h == 7:
                cb = (h // 4) * 128
                dst = bass.AP(
                    tensor=out.tensor, offset=cb,
                    ap=[[256, 128], [256 * 128, 8], [1, 128]])
                nc.sync.dma_start(out=dst, in_=out_sb[:, :, cb:cb + 128])

        for st in range(64 + PIPE):
            if st < 64:
                h, jm = st // 8, st % 8
                tp, s = h // 2, h % 2
                base = 64 * s
                if jm == 0:
                    psat_tiles[h] = psattp.tile(
                        [128, 8, 33], F32, tag="att", name=f"psat{h}")
                    if h % 2 == 0:
                        nc.vector.memset(psat_tiles[h], 0.0)
                    else:
                        nc.scalar.memzero(psat_tiles[h])
                psl = pslogp.tile([128, N], F32, tag="log", name=f"psl{st}")
                ew = ewp.tile([128, N], BF16, tag="ew", name=f"ew{st}")
                ew_tiles[st] = ew
                for half in range(2):
                    sl = slice(half * 512, (half + 1) * 512)
                    _lg(nc.tensor.matmul(
                        psl[:, sl], lhsT=kaug[base:base + 64, tp, jm, :],
                        rhs=qaug[base:base + 64, tp, sl],
                        start=True, stop=True), f"QK st{st} h{h} jm{jm}")
                # heads 2-7's SWT pipelines are sprinkled into the first
                # 12 steps (one half per step) while the ACT/DVE queues are
                # still shallow, then their gathers follow on SP
                if st < 12:
                    emit_swt_half(2 + st // 2, st % 2, "DA"[st % 2])
                if 12 <= st < 18:
                    emit_gather(st - 10, nc.sync)
                kind = EXP_PATTERN[st % len(EXP_PATTERN)]
                if kind == "A":
                    nc.scalar.activation(ew, psl, AF.Exp)
                else:
                    nc.vector.tensor_scalar(
                        out=ew.bitcast(I16), in0=psl,
                        scalar1=FE_A, scalar2=FE_B,
                        op0=mybir.AluOpType.mult, op1=mybir.AluOpType.add)
            if st >= PIPE:
                emit_av(st - PIPE)
                if (st - PIPE) % 8 == 7:
                    emit_norm((st - PIPE) // 8)


_NC_CACHE = {}


def _build():
    if "nc" in _NC_CACHE:
        return _NC_CACHE["nc"]
    nc = bass.Bass("TRN2", target_bir_lowering=False, debug=False,
                   enable_asserts=True, num_devices=8)
    ins = {
        "blob": nc.dram_tensor("blob", [128, 10368], BF16,
                               kind="ExternalInput").ap(),
    }
    outs = {
        "out": nc.dram_tensor("out", [N, 256], F32,
                              kind="ExternalOutput").ap(),
    }
    with tile.TileContext(nc) as tc:
        kernel_body(tc, outs, ins)
    split_multiwaits(nc)
    _NC_CACHE["nc"] = nc
    return nc


def _prep_core(xc, perm, C, rel_st):
    """Host-side layout prep for one core's batch. xc: [1024, 768] f32.

    Blob columns: [0:63 rel | 63:64 pad | 64:4160 qaug | 4160:8256 kaug |
    8256:10368 vws]."""
    q = (xc[:, 0:256] * SCALE).astype(BF)
    k = xc[:, 256:512].astype(BF)
    v = xc[:, 512:768].astype(BF)
    blob = np.zeros((128, 10368), BF)
    blob[:, 0:63] = rel_st
    qaug = blob[:, 64:4160].reshape(128, 4, N)
    kaug = blob[:, 4160:8256].reshape(128, 4, 8, 128)
    vws = blob[:, 8256:10368].reshape(128, 8, NH, 33)
    for tp in range(4):
        h0, h1 = 2 * tp, 2 * tp + 1
        qaug[0:32, tp] = q[:, 32 * h0:32 * h0 + 32].T
        qaug[32:64, tp] = q[perm, 32 * h0:32 * h0 + 32].T
        qaug[64:96, tp] = q[:, 32 * h1:32 * h1 + 32].T
        qaug[96:128, tp] = q[perm, 32 * h1:32 * h1 + 32].T
        kaug[0:32, tp] = k[:, 32 * h0:32 * h0 + 32].reshape(
            8, 128, 32).transpose(2, 0, 1)
        kaug[32:64, tp] = C
        kaug[64:96, tp] = k[:, 32 * h1:32 * h1 + 32].reshape(
            8, 128, 32).transpose(2, 0, 1)
        kaug[96:128, tp] = C
    vws[...] = 1.0
    vws[:, :, :, 0:32] = v.reshape(8, 128, NH, 32).transpose(1, 0, 2, 3)
    return blob


def kernel(inputs, key_rel_h, key_rel_w, _trace=False):
    nc = _build()
    x = np.asarray(inputs, dtype=np.float32).reshape(8, N, 768)
    relh = np.asarray(key_rel_h, dtype=np.float32)
    relw = np.asarray(key_rel_w, dtype=np.float32)

    idx = np.arange(N)
    perm = (idx % 32) * 32 + idx // 32
    rel_st = np.zeros((128, 63), BF)
    rel_st[0:32] = relw.T.astype(BF)
    rel_st[32:64] = relh.T.astype(BF)
    rel_st[64:128] = rel_st[0:64]
    t = np.arange(32)[:, None, None]
    jmv = np.arange(8)[None, :, None]
    mloc = np.arange(128)[None, None, :]
    C = (t == 4 * jmv + mloc // 32).astype(np.float32).astype(BF)

    in_maps = []
    for c in range(8):
        in_maps.append({"blob": _prep_core(x[c], perm, C, rel_st)})
    res = bass_utils.run_bass_kernel_spmd(
        nc, in_maps, core_ids=list(range(8)), trace=_trace)
    outp = np.stack([r["out"] for r in res.results])
    if _trace:
        kernel.last_results = res
    return outp.reshape(8, 32, 32, 256)
